# revision 22
# baseline (speedup 1.0000x reference)
"""GAT layer (nn_GAT_10754598109596) Bass kernel for 8 TRN2 NeuronCores.

Sharding: data-parallel over batch B=8, one batch element per core.

Per-core algorithm (N=2048 nodes, d=128 feat, all f32):
  setup (once): Wh = h @ W; f1 = h @ (W a1) (per-chunk columns);
                f2b = broadcast(f2 = h @ (W a2)) over partitions [128, N]
  per 128-row q-tile i:
    s      = (1000*adj - 1000) + f2b      (GpSimd cast-affine + DVE add;
             the mask addend is exactly 0 on edges so edge values are exact)
    L      = prelu(s + f1[q], alpha=0.2)  (ScalarE, bias=f1 per-partition)
    E      = exp(L), S[q] = rowsum(E)     (ScalarE + accum_out; non-edges
             underflow to exactly 0, matching the reference's hard zeros)
    A      = E * (1/S[q])                 (VectorE 2x) -> DMA attention rows
    A.T    chunks via PE transpose -> PSUM -> copyback (ScalarE/VectorE
             alternating) into a 2-tile pair buffer
    per pair: h'.T[d, 2x128 q] = sum_j Wh[j]-stationary @ A.T-pair (N=256
             moving, LDWEIGHTS hidden), transpose back, out = h + h' -> DMA

The walrus build in this container supports at most ONE semaphore wait per
instruction (none on Drain); split_waits() post-processes the Tile-generated
BIR to hoist extra waits into standalone EventSemaphore instructions.
"""

import sys
import types
from contextlib import ExitStack

import numpy as np

import concourse.bass as bass
import concourse.tile as tile
from concourse import mybir
from concourse.bass_utils import run_bass_kernel_spmd
from concourse.masks import make_identity


def _register_ntff_hook():
    """The trimmed antenv in this container lacks axon_hooks, so boot()
    silently skips NTFF-profile registration. Shim the module and register
    the ctypes-based hook so trace=True produces HW profiles."""
    if "antenv.axon_hooks" in sys.modules:
        return
    mod = types.ModuleType("antenv.axon_hooks")
    holder = {"hook": None}
    mod.set_axon_ntff_profile_hook = lambda h: holder.__setitem__("hook", h)
    mod.get_axon_ntff_profile_hook = lambda: holder["hook"]
    sys.modules["antenv.axon_hooks"] = mod
    try:
        from trn_agent_boot.trn_boot import _ntff_profile_via_ctypes

        mod.set_axon_ntff_profile_hook(
            _ntff_profile_via_ctypes("/opt/axon/libaxon_pjrt.so")
        )
    except Exception:
        pass


try:
    _register_ntff_hook()
except Exception:
    pass

B, N, D = 8, 2048, 128
NT = N // 128  # 16 q-tiles / k-chunks
NQ = NT // 4   # 4 quads
ALPHA = 0.2
MASKC = 1000.0
F32 = mybir.dt.float32
I32 = mybir.dt.int32
AF = mybir.ActivationFunctionType
OP = mybir.AluOpType


def split_waits(nc, limit_default=1, limit_by_op={"Drain": 0, "Halt": 0}):
    """Walrus here allows <=1 sem-wait per instruction (0 on Drain). Hoist
    extra waits into standalone EventSemaphore instructions on the same
    engine, inserted immediately before the owning instruction."""
    n_split = 0
    for f in nc.m.functions:
        for bb in f.blocks:
            out = []
            for inst in bb.instructions:
                si = inst.sync_info
                waits = list(si.on_wait) if (si and si.on_wait) else []
                opname = getattr(inst, "opcode", None) or type(inst).__name__
                limit = limit_by_op.get(str(opname).replace("Inst", ""), limit_default)
                if len(waits) > limit:
                    keep, extra = waits[:limit], waits[limit:]
                    for w in extra:
                        wi = mybir.InstEventSemaphore(
                            name=f"{inst.name}-w{n_split}",
                            ins=[], outs=[],
                            sync_info=mybir.SyncInfo(on_wait=[w], on_update=[]),
                        )
                        wi.engine = inst.engine
                        out.append(wi)
                        n_split += 1
                    si.on_wait = keep
                out.append(inst)
            bb.instructions = out
    return nc


def build_kernel():
    nc = bass.Bass("TRN2", target_bir_lowering=False, debug=False)

    h = nc.dram_tensor("h", [N, D], F32, kind="ExternalInput")
    adj = nc.dram_tensor("adj", [N, N], I32, kind="ExternalInput")
    w = nc.dram_tensor("w", [D, D], F32, kind="ExternalInput")
    a = nc.dram_tensor("a", [2 * D, 1], F32, kind="ExternalInput")
    attn = nc.dram_tensor("attn", [N, N], F32, kind="ExternalOutput")
    out = nc.dram_tensor("out", [N, D], F32, kind="ExternalOutput")

    with tile.TileContext(nc) as tc, ExitStack() as ctx:
        const = ctx.enter_context(tc.tile_pool(name="const", bufs=1))

        ident = const.tile([128, 128], F32)
        make_identity(nc, ident[:])

        # ---- load inputs -------------------------------------------------
        w_sb = const.tile([128, 128], F32)       # [din, dout]
        nc.sync.dma_start(w_sb[:], bass.AP(w, 0, [[128, 128], [1, 128]]))
        a_sb = const.tile([128, 2], F32)         # a1 | a2, partition = dout
        nc.sync.dma_start(a_sb[:], bass.AP(a, 0, [[1, 128], [128, 2]]))
        # h chunk-major: h_sb[p, 128*j + d] = h[128*j + p, d]
        h_sb = const.tile([128, N], F32)
        nc.sync.dma_start(h_sb[:], bass.AP(h, 0, [[128, 128], [128 * 128, NT], [1, 128]]))

        hT_sb = const.tile([128, N], F32)        # hT_sb[d, 128*j + p] = h[128*j+p, d]
        wT_sb = const.tile([128, 128], F32)      # W.T: [dout, din]
        c_sb = const.tile([128, 2], F32)         # c1|c2 = W @ a1|a2, partition = din
        c2b_sb = const.tile([128, 128], F32)     # c2 broadcast along free
        wh_sb = const.tile([128, N], F32)        # Wh chunk-major [k_sub, d]
        fc_sb = const.tile([128, 2 * NT], F32)   # per chunk j: cols 2j=f1, 2j+1=f2
        f2b_sb = const.tile([128, N], F32)       # f2[k] broadcast over partitions
        wc_sb = const.tile([128, 130], F32)      # [W | c] concat

        with tc.tile_pool(name="setup_ps", bufs=2, space="PSUM") as sps, \
             tc.tile_pool(name="setup_ps1", bufs=1, space="PSUM") as sps1:
            # W.T, c, c2b (only need W/a DMAs - do first)
            trw = sps1.tile([128, 128], F32, tag="trw")
            nc.tensor.transpose(trw[:], w_sb[:], ident[:])
            nc.vector.tensor_copy(wT_sb[:], trw[:])
            cps = sps1.tile([128, 2], F32, tag="cps")
            nc.tensor.matmul(cps[:], wT_sb[:], a_sb[:], start=True, stop=True)
            nc.vector.tensor_copy(c_sb[:], cps[:])
            nc.vector.tensor_scalar(
                c2b_sb[:], ident[:], 0.0, c_sb[:, 1:2], OP.mult, OP.add,
            )
            # wc = [W | c] for fused Wh+f1+f2 matmuls later
            nc.vector.tensor_copy(wc_sb[:, 0:128], w_sb[:])
            nc.vector.tensor_copy(wc_sb[:, 128:130], c_sb[:])
            # interleave h-chunk transposes with f2b blocks so f2b (which
            # gates every tile's elementwise chain) completes ASAP
            for g in range(NT // 4):
                tr = sps.tile([128, 512], F32, tag="tr")
                for j4 in range(4):
                    j = 4 * g + j4
                    nc.tensor.transpose(
                        tr[:, 128 * j4:128 * (j4 + 1)],
                        h_sb[:, 128 * j:128 * (j + 1)], ident[:],
                    )
                nc.vector.tensor_copy(hT_sb[:, 512 * g:512 * (g + 1)], tr[:])
                f2p = sps.tile([128, 512], F32, tag="f2p")
                nc.tensor.matmul(
                    f2p[:], c2b_sb[:], hT_sb[:, 512 * g:512 * (g + 1)],
                    start=True, stop=True,
                )
                nc.scalar.copy(f2b_sb[:, 512 * g:512 * (g + 1)], f2p[:])
            # Wh chunks + f columns fused: [Wh | f1 | f2] = hT.T @ [W | c]
            fcp = sps1.tile([128, 2 * NT], F32, tag="fcp")
            for g in range(NT // 4):
                whp = sps.tile([128, 512], F32, tag="tr")
                for j4 in range(4):
                    j = 4 * g + j4
                    nc.tensor.matmul(
                        whp[:, 128 * j4:128 * (j4 + 1)],
                        hT_sb[:, 128 * j:128 * (j + 1)], wc_sb[:, 0:128],
                        start=True, stop=True,
                    )
                    nc.tensor.matmul(
                        fcp[:, 2 * j:2 * (j + 1)],
                        hT_sb[:, 128 * j:128 * (j + 1)], wc_sb[:, 128:130],
                        start=True, stop=True,
                    )
                nc.scalar.copy(wh_sb[:, 512 * g:512 * (g + 1)], whp[:])
            nc.vector.tensor_copy(fc_sb[:], fcp[:])

        # ---- main loop over q-tiles -------------------------------------
        with ExitStack() as lctx:
            p_adj = lctx.enter_context(tc.tile_pool(name="p_adj", bufs=5))
            p_s = lctx.enter_context(tc.tile_pool(name="p_s", bufs=6))
            p_TAQ = lctx.enter_context(tc.tile_pool(name="p_TAQ", bufs=2))
            p_small = lctx.enter_context(tc.tile_pool(name="p_small", bufs=4))
            p_hpt = lctx.enter_context(tc.tile_pool(name="p_hpt", bufs=2))
            p_o = lctx.enter_context(tc.tile_pool(name="p_o", bufs=2))
            p_taps = lctx.enter_context(tc.tile_pool(name="p_taps", bufs=4, space="PSUM"))
            p_hpps = lctx.enter_context(tc.tile_pool(name="p_hpps", bufs=2, space="PSUM"))
            p_hp2ps = lctx.enter_context(tc.tile_pool(name="p_hp2ps", bufs=2, space="PSUM"))

            TAQ = None
            for i in range(NT):
                row0 = 128 * i
                q = i // 2   # pair index
                t = i % 2    # slot within pair
                if t == 0:
                    TAQ = p_TAQ.tile([128, 2 * N], F32)  # tile-major pair

                adj_i = p_adj.tile([128, N], I32)
                nc.sync.dma_start(
                    adj_i[:], bass.AP(adj, row0 * N, [[N, 128], [1, N]])
                )
                # s = 1000*adj - 1000 (GpSimd cast), then += f2b (DVE)
                s = p_s.tile([128, N], F32)
                nc.gpsimd.tensor_scalar(
                    s[:], adj_i[:], MASKC, -MASKC, OP.mult, OP.add,
                )
                nc.vector.tensor_tensor(s[:], s[:], f2b_sb[:], OP.add)

                # L = prelu(s + f1[q])  (in-place over s)
                nc.scalar.activation(
                    s[:], s[:], AF.Prelu,
                    bias=fc_sb[:, 2 * i:2 * i + 1], scale=1.0, alpha=ALPHA,
                )
                # E = exp(L), S = rowsum(E)  (in-place)
                S = p_small.tile([128, 1], F32, tag="S")
                nc.scalar.activation(
                    s[:], s[:], AF.Exp, bias=0.0, scale=1.0, accum_out=S[:],
                )
                rS = p_small.tile([128, 1], F32, tag="rS")
                nc.vector.reciprocal(rS[:], S[:])

                # A = E / S  (normalized attention row block, in-place)
                A = s
                nc.vector.tensor_scalar(A[:], A[:], rS[:], None, OP.mult)
                nc.sync.dma_start(
                    bass.AP(attn, row0 * N, [[N, 128], [1, N]]), A[:]
                )

                # A.T chunks -> TAQ slot t (tile-major: offset N*t + 128*j)
                for g in range(4):
                    ta_ps = p_taps.tile([128, 512], F32, tag="ta_ps")
                    for j4 in range(4):
                        j = 4 * g + j4
                        nc.tensor.transpose(
                            ta_ps[:, 128 * j4:128 * (j4 + 1)],
                            A[:, 128 * j:128 * (j + 1)], ident[:],
                        )
                    dst = TAQ[:, N * t + 512 * g:N * t + 512 * (g + 1)]
                    if g % 2 == 0:
                        nc.scalar.copy(dst, ta_ps[:])
                    else:
                        nc.vector.tensor_copy(dst, ta_ps[:])

                if t == 1:
                    # h'.T [d, 2x128] = sum_j Wh[j].T @ A.T-pair chunk j
                    hpT = p_hpps.tile([128, 256], F32, tag="hpT")
                    taq_r = TAQ[:].rearrange("p (t f) -> p t f", t=2)
                    for j in range(NT):
                        rhs = taq_r[:, :, 128 * j:128 * (j + 1)]
                        nc.tensor.matmul(
                            hpT[:], wh_sb[:, 128 * j:128 * (j + 1)], rhs,
                            start=(j == 0), stop=(j == NT - 1),
                        )
                    hpT_sb = p_hpt.tile([128, 256], F32)
                    nc.vector.tensor_copy(hpT_sb[:], hpT[:])
                    hp2 = p_hp2ps.tile([128, 256], F32, tag="hp2")
                    for u in range(2):
                        nc.tensor.transpose(
                            hp2[:, 128 * u:128 * (u + 1)],
                            hpT_sb[:, 128 * u:128 * (u + 1)], ident[:],
                        )
                    o_sb = p_o.tile([128, 256], F32)
                    nc.vector.tensor_tensor(
                        o_sb[:], hp2[:], h_sb[:, 256 * q:256 * (q + 1)], OP.add,
                    )
                    nc.sync.dma_start(
                        bass.AP(out, 256 * q * D, [[D, 128], [128 * D, 2], [1, D]]),
                        o_sb[:],
                    )

    split_waits(nc)
    return nc


_cache = {}
last_perf = {}


def kernel(h, adj, W, a, trace=False):
    if "nc" not in _cache:
        _cache["nc"] = build_kernel()
    nc = _cache["nc"]

    h = np.ascontiguousarray(h, dtype=np.float32)
    adj = np.ascontiguousarray(adj, dtype=np.int32)
    W = np.ascontiguousarray(W, dtype=np.float32)
    a = np.ascontiguousarray(a, dtype=np.float32)

    in_maps = [
        {"h": h[b], "adj": adj[b], "w": W, "a": a} for b in range(B)
    ]
    last_exc = None
    for attempt in range(4):
        try:
            res = run_bass_kernel_spmd(
                nc, in_maps, core_ids=list(range(B)), trace=trace,
            )
            # fetch eagerly inside the retry: device errors surface lazily
            # when the PJRT arrays are first read
            out = np.stack(
                [np.asarray(res.results[b]["out"]) for b in range(B)]
            ).astype(np.float32, copy=False)
            attention = np.stack(
                [np.asarray(res.results[b]["attn"]) for b in range(B)]
            ).astype(np.float32, copy=False)
            last_perf["exec_time_ns"] = res.exec_time_ns
            last_perf["trace"] = (
                res.instructions_and_trace[1] if res.instructions_and_trace else None
            )
            last_perf["profile_json"] = res.profile_json
            return out, attention
        except Exception as e:  # transient NRT_EXEC_UNIT_UNRECOVERABLE etc.
            last_exc = e
    raise last_exc


# revision 23
# speedup vs baseline: 1.0344x; 1.0344x over previous
"""GAT layer (nn_GAT_10754598109596) Bass kernel for 8 TRN2 NeuronCores.

Sharding: data-parallel over batch B=8, one batch element per core.

Per-core algorithm (N=2048 nodes, d=128 feat, all f32):
  setup (once): Wh = h @ W; f1 = h @ (W a1) (per-chunk columns);
                f2b = broadcast(f2 = h @ (W a2)) over partitions [128, N]
  per 128-row q-tile i:
    s      = (1000*adj - 1000) + f2b      (GpSimd cast-affine + DVE add;
             the mask addend is exactly 0 on edges so edge values are exact)
    L      = prelu(s + f1[q], alpha=0.2)  (ScalarE, bias=f1 per-partition)
    E      = exp(L), S[q] = rowsum(E)     (ScalarE + accum_out; non-edges
             underflow to exactly 0, matching the reference's hard zeros)
    A      = E * (1/S[q])                 (VectorE 2x) -> DMA attention rows
    A.T    chunks via PE transpose -> PSUM -> copyback (ScalarE/VectorE
             alternating) into a 2-tile pair buffer
    per pair: h'.T[d, 2x128 q] = sum_j Wh[j]-stationary @ A.T-pair (N=256
             moving, LDWEIGHTS hidden), transpose back, out = h + h' -> DMA

The walrus build in this container supports at most ONE semaphore wait per
instruction (none on Drain); split_waits() post-processes the Tile-generated
BIR to hoist extra waits into standalone EventSemaphore instructions.
"""

import sys
import types
from contextlib import ExitStack

import numpy as np

import concourse.bass as bass
import concourse.tile as tile
from concourse import mybir
from concourse.bass_utils import run_bass_kernel_spmd
from concourse.masks import make_identity


def _register_ntff_hook():
    """The trimmed antenv in this container lacks axon_hooks, so boot()
    silently skips NTFF-profile registration. Shim the module and register
    the ctypes-based hook so trace=True produces HW profiles."""
    if "antenv.axon_hooks" in sys.modules:
        return
    mod = types.ModuleType("antenv.axon_hooks")
    holder = {"hook": None}
    mod.set_axon_ntff_profile_hook = lambda h: holder.__setitem__("hook", h)
    mod.get_axon_ntff_profile_hook = lambda: holder["hook"]
    sys.modules["antenv.axon_hooks"] = mod
    try:
        from trn_agent_boot.trn_boot import _ntff_profile_via_ctypes

        mod.set_axon_ntff_profile_hook(
            _ntff_profile_via_ctypes("/opt/axon/libaxon_pjrt.so")
        )
    except Exception:
        pass


try:
    _register_ntff_hook()
except Exception:
    pass

B, N, D = 8, 2048, 128
NT = N // 128  # 16 q-tiles / k-chunks
NQ = NT // 4   # 4 quads
ALPHA = 0.2
MASKC = 1000.0
F32 = mybir.dt.float32
I32 = mybir.dt.int32
AF = mybir.ActivationFunctionType
OP = mybir.AluOpType


def split_waits(nc, limit_default=1, limit_by_op={"Drain": 0, "Halt": 0}):
    """Walrus here allows <=1 sem-wait per instruction (0 on Drain). Hoist
    extra waits into standalone EventSemaphore instructions on the same
    engine, inserted immediately before the owning instruction."""
    n_split = 0
    for f in nc.m.functions:
        for bb in f.blocks:
            out = []
            for inst in bb.instructions:
                si = inst.sync_info
                waits = list(si.on_wait) if (si and si.on_wait) else []
                opname = getattr(inst, "opcode", None) or type(inst).__name__
                limit = limit_by_op.get(str(opname).replace("Inst", ""), limit_default)
                if len(waits) > limit:
                    keep, extra = waits[:limit], waits[limit:]
                    for w in extra:
                        wi = mybir.InstEventSemaphore(
                            name=f"{inst.name}-w{n_split}",
                            ins=[], outs=[],
                            sync_info=mybir.SyncInfo(on_wait=[w], on_update=[]),
                        )
                        wi.engine = inst.engine
                        out.append(wi)
                        n_split += 1
                    si.on_wait = keep
                out.append(inst)
            bb.instructions = out
    return nc


def build_kernel():
    nc = bass.Bass("TRN2", target_bir_lowering=False, debug=False)

    h = nc.dram_tensor("h", [N, D], F32, kind="ExternalInput")
    adj = nc.dram_tensor("adj", [N, N], I32, kind="ExternalInput")
    w = nc.dram_tensor("w", [D, D], F32, kind="ExternalInput")
    a = nc.dram_tensor("a", [2 * D, 1], F32, kind="ExternalInput")
    attn = nc.dram_tensor("attn", [N, N], F32, kind="ExternalOutput")
    out = nc.dram_tensor("out", [N, D], F32, kind="ExternalOutput")

    with tile.TileContext(nc) as tc, ExitStack() as ctx:
        const = ctx.enter_context(tc.tile_pool(name="const", bufs=1))

        ident = const.tile([128, 128], F32)
        make_identity(nc, ident[:])

        # ---- load inputs -------------------------------------------------
        w_sb = const.tile([128, 128], F32)       # [din, dout]
        nc.sync.dma_start(w_sb[:], bass.AP(w, 0, [[128, 128], [1, 128]]))
        a_sb = const.tile([128, 2], F32)         # a1 | a2, partition = dout
        nc.sync.dma_start(a_sb[:], bass.AP(a, 0, [[1, 128], [128, 2]]))
        # h chunk-major: h_sb[p, 128*j + d] = h[128*j + p, d]
        h_sb = const.tile([128, N], F32)
        nc.sync.dma_start(h_sb[:], bass.AP(h, 0, [[128, 128], [128 * 128, NT], [1, 128]]))

        hT_sb = const.tile([128, N], F32)        # hT_sb[d, 128*j + p] = h[128*j+p, d]
        wT_sb = const.tile([128, 128], F32)      # W.T: [dout, din]
        c_sb = const.tile([128, 2], F32)         # c1|c2 = W @ a1|a2, partition = din
        c2b_sb = const.tile([128, 128], F32)     # c2 broadcast along free
        wh_sb = const.tile([128, N], F32)        # Wh chunk-major [k_sub, d]
        fc_sb = const.tile([128, 2 * NT], F32)   # per chunk j: cols 2j=f1, 2j+1=f2
        f2b_sb = const.tile([128, N], F32)       # f2[k] broadcast over partitions
        wc_sb = const.tile([128, 130], F32)      # [W | c] concat

        with tc.tile_pool(name="setup_ps", bufs=2, space="PSUM") as sps, \
             tc.tile_pool(name="setup_ps1", bufs=1, space="PSUM") as sps1:
            # W.T, c, c2b (only need W/a DMAs - do first)
            trw = sps1.tile([128, 128], F32, tag="trw")
            nc.tensor.transpose(trw[:], w_sb[:], ident[:])
            nc.vector.tensor_copy(wT_sb[:], trw[:])
            cps = sps1.tile([128, 2], F32, tag="cps")
            nc.tensor.matmul(cps[:], wT_sb[:], a_sb[:], start=True, stop=True)
            nc.vector.tensor_copy(c_sb[:], cps[:])
            nc.vector.tensor_scalar(
                c2b_sb[:], ident[:], 0.0, c_sb[:, 1:2], OP.mult, OP.add,
            )
            # wc = [W | c] for fused Wh+f1+f2 matmuls later
            nc.vector.tensor_copy(wc_sb[:, 0:128], w_sb[:])
            nc.vector.tensor_copy(wc_sb[:, 128:130], c_sb[:])
            # interleave h-chunk transposes with f2b blocks so f2b (which
            # gates every tile's elementwise chain) completes ASAP
            for g in range(NT // 4):
                tr = sps.tile([128, 512], F32, tag="tr")
                for j4 in range(4):
                    j = 4 * g + j4
                    nc.tensor.transpose(
                        tr[:, 128 * j4:128 * (j4 + 1)],
                        h_sb[:, 128 * j:128 * (j + 1)], ident[:],
                    )
                nc.vector.tensor_copy(hT_sb[:, 512 * g:512 * (g + 1)], tr[:])
                f2p = sps.tile([128, 512], F32, tag="f2p")
                nc.tensor.matmul(
                    f2p[:], c2b_sb[:], hT_sb[:, 512 * g:512 * (g + 1)],
                    start=True, stop=True,
                )
                nc.scalar.copy(f2b_sb[:, 512 * g:512 * (g + 1)], f2p[:])
            # Wh chunks + f columns fused: [Wh | f1 | f2] = hT.T @ [W | c]
            fcp = sps1.tile([128, 2 * NT], F32, tag="fcp")
            for g in range(NT // 4):
                whp = sps.tile([128, 512], F32, tag="tr")
                for j4 in range(4):
                    j = 4 * g + j4
                    nc.tensor.matmul(
                        whp[:, 128 * j4:128 * (j4 + 1)],
                        hT_sb[:, 128 * j:128 * (j + 1)], wc_sb[:, 0:128],
                        start=True, stop=True,
                    )
                    nc.tensor.matmul(
                        fcp[:, 2 * j:2 * (j + 1)],
                        hT_sb[:, 128 * j:128 * (j + 1)], wc_sb[:, 128:130],
                        start=True, stop=True,
                    )
                nc.vector.tensor_copy(wh_sb[:, 512 * g:512 * (g + 1)], whp[:])
            nc.vector.tensor_copy(fc_sb[:], fcp[:])

        # ---- main loop over q-tiles -------------------------------------
        with ExitStack() as lctx:
            p_adj = lctx.enter_context(tc.tile_pool(name="p_adj", bufs=5))
            p_s = lctx.enter_context(tc.tile_pool(name="p_s", bufs=6))
            p_TAQ = lctx.enter_context(tc.tile_pool(name="p_TAQ", bufs=2))
            p_small = lctx.enter_context(tc.tile_pool(name="p_small", bufs=4))
            p_hpt = lctx.enter_context(tc.tile_pool(name="p_hpt", bufs=2))
            p_o = lctx.enter_context(tc.tile_pool(name="p_o", bufs=2))
            p_taps = lctx.enter_context(tc.tile_pool(name="p_taps", bufs=4, space="PSUM"))
            p_hpps = lctx.enter_context(tc.tile_pool(name="p_hpps", bufs=2, space="PSUM"))
            p_hp2ps = lctx.enter_context(tc.tile_pool(name="p_hp2ps", bufs=2, space="PSUM"))

            TAQ = None
            for i in range(NT):
                row0 = 128 * i
                q = i // 2   # pair index
                t = i % 2    # slot within pair
                if t == 0:
                    TAQ = p_TAQ.tile([128, 2 * N], F32)  # tile-major pair

                adj_i = p_adj.tile([128, N], I32)
                nc.sync.dma_start(
                    adj_i[:], bass.AP(adj, row0 * N, [[N, 128], [1, N]])
                )
                # s = 1000*adj - 1000 (GpSimd cast), then += f2b (DVE)
                s = p_s.tile([128, N], F32)
                nc.gpsimd.tensor_scalar(
                    s[:], adj_i[:], MASKC, -MASKC, OP.mult, OP.add,
                )
                nc.vector.tensor_tensor(s[:], s[:], f2b_sb[:], OP.add)

                # L = prelu(s + f1[q])  (in-place over s)
                nc.scalar.activation(
                    s[:], s[:], AF.Prelu,
                    bias=fc_sb[:, 2 * i:2 * i + 1], scale=1.0, alpha=ALPHA,
                )
                # E = exp(L), S = rowsum(E)  (in-place)
                S = p_small.tile([128, 1], F32, tag="S")
                nc.scalar.activation(
                    s[:], s[:], AF.Exp, bias=0.0, scale=1.0, accum_out=S[:],
                )
                rS = p_small.tile([128, 1], F32, tag="rS")
                nc.vector.reciprocal(rS[:], S[:])

                # A = E / S  (normalized attention row block, in-place)
                A = s
                nc.vector.tensor_scalar(A[:], A[:], rS[:], None, OP.mult)
                nc.sync.dma_start(
                    bass.AP(attn, row0 * N, [[N, 128], [1, N]]), A[:]
                )

                # A.T chunks -> TAQ slot t (tile-major: offset N*t + 128*j)
                for g in range(4):
                    ta_ps = p_taps.tile([128, 512], F32, tag="ta_ps")
                    for j4 in range(4):
                        j = 4 * g + j4
                        nc.tensor.transpose(
                            ta_ps[:, 128 * j4:128 * (j4 + 1)],
                            A[:, 128 * j:128 * (j + 1)], ident[:],
                        )
                    dst = TAQ[:, N * t + 512 * g:N * t + 512 * (g + 1)]
                    if g % 2 == 0:
                        nc.scalar.copy(dst, ta_ps[:])
                    else:
                        nc.vector.tensor_copy(dst, ta_ps[:])

                if t == 1:
                    # h'.T [d, 2x128] = sum_j Wh[j].T @ A.T-pair chunk j
                    hpT = p_hpps.tile([128, 256], F32, tag="hpT")
                    taq_r = TAQ[:].rearrange("p (t f) -> p t f", t=2)
                    for j in range(NT):
                        rhs = taq_r[:, :, 128 * j:128 * (j + 1)]
                        nc.tensor.matmul(
                            hpT[:], wh_sb[:, 128 * j:128 * (j + 1)], rhs,
                            start=(j == 0), stop=(j == NT - 1),
                        )
                    hpT_sb = p_hpt.tile([128, 256], F32)
                    nc.vector.tensor_copy(hpT_sb[:], hpT[:])
                    hp2 = p_hp2ps.tile([128, 256], F32, tag="hp2")
                    for u in range(2):
                        nc.tensor.transpose(
                            hp2[:, 128 * u:128 * (u + 1)],
                            hpT_sb[:, 128 * u:128 * (u + 1)], ident[:],
                        )
                    o_sb = p_o.tile([128, 256], F32)
                    nc.vector.tensor_tensor(
                        o_sb[:], hp2[:], h_sb[:, 256 * q:256 * (q + 1)], OP.add,
                    )
                    nc.sync.dma_start(
                        bass.AP(out, 256 * q * D, [[D, 128], [128 * D, 2], [1, D]]),
                        o_sb[:],
                    )

    split_waits(nc)
    return nc


_cache = {}
last_perf = {}


def kernel(h, adj, W, a, trace=False):
    if "nc" not in _cache:
        _cache["nc"] = build_kernel()
    nc = _cache["nc"]

    h = np.ascontiguousarray(h, dtype=np.float32)
    adj = np.ascontiguousarray(adj, dtype=np.int32)
    W = np.ascontiguousarray(W, dtype=np.float32)
    a = np.ascontiguousarray(a, dtype=np.float32)

    in_maps = [
        {"h": h[b], "adj": adj[b], "w": W, "a": a} for b in range(B)
    ]
    last_exc = None
    for attempt in range(4):
        try:
            res = run_bass_kernel_spmd(
                nc, in_maps, core_ids=list(range(B)), trace=trace,
            )
            # fetch eagerly inside the retry: device errors surface lazily
            # when the PJRT arrays are first read
            out = np.stack(
                [np.asarray(res.results[b]["out"]) for b in range(B)]
            ).astype(np.float32, copy=False)
            attention = np.stack(
                [np.asarray(res.results[b]["attn"]) for b in range(B)]
            ).astype(np.float32, copy=False)
            last_perf["exec_time_ns"] = res.exec_time_ns
            last_perf["trace"] = (
                res.instructions_and_trace[1] if res.instructions_and_trace else None
            )
            last_perf["profile_json"] = res.profile_json
            return out, attention
        except Exception as e:  # transient NRT_EXEC_UNIT_UNRECOVERABLE etc.
            last_exc = e
    raise last_exc
